# revision 43
# baseline (speedup 1.0000x reference)
"""Trainium2 Bass kernel for the 2-layer TransformerConv GNN (edge-parallel, 8 cores).

Strategy (edge parallel, per sharding hint):
  - Sort edges by dst; shard nodes into 8 equal slices of 1250; each core owns
    all edges whose dst falls in its slice, so segment-softmax and scatter-add
    are core-local (no softmax-stat collectives needed).
  - Layer-1 K/V node projections are computed replicated on every core
    (input x is replicated); layer-2 K/V are data-parallel over nodes followed
    by an AllGather of the fused K|V table. BN statistics use a tiny AllReduce.
  - Per core, edges are grouped into 64-node blocks; each block's <=9 tiles of
    128 edges accumulate their segment-sums in PSUM via a 0/1 selection-matrix
    matmul, so no indirect scatter is needed.
  - Per 128-edge tile: K|V rows are gathered with one indirect DMA from the
    fused [N,512] table, q rows (+ precomputed t = (q . We)/sqrt(C)) with a
    second indirect DMA. Softmax skips the max-subtraction (logits are O(1))
    and normalization is deferred to the node phase:
       out_n = (sum_e ex_e * v_e + (sum_e ex_e*ea_e) * We) / (sum_e ex_e + 1e-16)
"""

import math

import numpy as np

N, E, H, C, D, F = 10000, 160000, 2, 128, 256, 128
R, NS, BLK = 8, 1250, 64
NB = (NS + BLK - 1) // BLK          # 20 blocks/core
NSPAD = NB * BLK                    # 1280
NFULL = 10112                       # 79*128, padded global node count
NT1 = NFULL // 128                  # 79
NTS = NSPAD // 128                  # 10
EPS = 1e-5
INV = 1.0 / math.sqrt(C)

_CACHE = {}


def _prepare(edge_index, edge_attr):
    """Host-side index preprocessing -> per-core tile arrays (data only)."""
    src = edge_index[0].astype(np.int32)
    dst = edge_index[1].astype(np.int32)
    ea = edge_attr[:, 0].astype(np.float32)
    perm = np.argsort(dst, kind="stable")
    sdst, ssrc, sea = dst[perm], src[perm], ea[perm]
    bounds = np.searchsorted(sdst, np.arange(0, N + 1, NS))
    core_info = []
    cnts = np.zeros((R, NB), dtype=np.int64)
    for r in range(R):
        lo, hi = bounds[r], bounds[r + 1]
        ldst = sdst[lo:hi] - NS * r
        bb = np.searchsorted(ldst, np.arange(0, NSPAD + 1, BLK))
        cnts[r] = np.diff(bb)
        core_info.append((lo, bb))
    n_b = np.maximum(1, np.ceil(cnts.max(axis=0) / 128).astype(np.int64))
    T = int(n_b.sum())
    per_core = []
    for r in range(R):
        lo, bb = core_info[r]
        SRC = np.zeros((T, 128), np.int32)
        DST = np.zeros((T, 128), np.int32)
        EAV = np.zeros((T, 128), np.float32)
        S = np.zeros((T, 128, BLK), np.float32)
        t = 0
        for b in range(NB):
            base = lo + bb[b]
            nb_edges = bb[b + 1] - bb[b]
            for i in range(n_b[b]):
                e0 = min(128 * i, nb_edges)
                e1 = min(128 * (i + 1), nb_edges)
                cnt = e1 - e0
                if cnt > 0:
                    sl = slice(base + e0, base + e1)
                    ld = sdst[sl] - NS * r
                    SRC[t, :cnt] = ssrc[sl]
                    DST[t, :cnt] = ld
                    EAV[t, :cnt] = sea[sl]
                    S[t, np.arange(cnt), ld - BLK * b] = 1.0
                DST[t, cnt:] = BLK * b
                t += 1
        import ml_dtypes
        bf16 = ml_dtypes.bfloat16
        Sne = np.zeros((T, 128, 128), np.float32)
        Sne[:, :BLK, :] = S.transpose(0, 2, 1)
        onesea = np.zeros((128, 2 * T), np.float32)
        onesea[:, 0::2] = 1.0
        onesea[:, 1::2] = EAV.T
        per_core.append(
            dict(
                SRC=np.ascontiguousarray(SRC.T),                       # [128, T]
                EAV=np.ascontiguousarray(EAV.T),                       # [128, T]
                SEN=np.ascontiguousarray(
                    S.transpose(1, 0, 2).reshape(128, T * BLK)).astype(bf16),
                SNE=np.ascontiguousarray(
                    Sne.transpose(1, 0, 2).reshape(128, T * 128)).astype(bf16),
                OSEA=np.ascontiguousarray(onesea).astype(bf16),
            )
        )
    return per_core, n_b, T


def _build(T, n_b, stage=99, zero_bias=False):
    """Build + schedule the (shared-across-cores) Bass program.

    stage: 1=L1 projections, 2=+edge1, 3=+node1 (AllReduce), 4=+proj2 (AllGather),
           5=+edge2, 99=full.
    """
    import concourse.bass as bass
    import concourse.mybir as mybir
    import concourse.tile as tile
    from concourse import bacc
    from concourse.masks import make_identity

    f32 = mybir.dt.float32
    bf16 = mybir.dt.bfloat16
    i32 = mybir.dt.int32
    Alu = mybir.AluOpType
    Act = mybir.ActivationFunctionType
    AX = mybir.AxisListType

    nc = bacc.Bacc("TRN2", target_bir_lowering=False, debug=False,
                   enable_asserts=False, num_devices=R)

    def din(name, shape, dtype=f32):
        return nc.dram_tensor(name, shape, dtype, kind="ExternalInput").ap()

    XTF = din("XTF", [128, NFULL], bf16)
    XTS = din("XTS", [128, NSPAD], bf16)
    SRC = din("SRC", [128, T], i32)
    EAV = din("EAV", [128, T])
    SEN = din("SEN", [128, T * BLK], bf16)
    SNE = din("SNE", [128, T * 128], bf16)
    OSEA = din("OSEA", [128, 2 * T], bf16)
    W1 = {k: din(k, [128, 256], bf16) for k in ("WK1", "WV1", "WQ1", "WS1")}
    B1 = {k: din(k, [128, 256]) for k in ("BK1", "BV1", "BQ1", "BS1")}
    WE1 = din("WE1", [128, 256])
    W2 = {k: din(k, [128, 512], bf16) for k in ("WK2", "WV2", "WQ2", "WS2")}
    B2 = {k: din(k, [128, 256]) for k in ("BK2", "BV2", "BQ2", "BS2")}
    WE2 = din("WE2", [128, 256])
    G1T, B1TT = din("G1T", [128, 2]), din("B1TT", [128, 2])
    G2T, B2TT = din("G2T", [128, 2]), din("B2TT", [128, 2])
    OUTT = nc.dram_tensor("OUTT", [256, NSPAD], f32, kind="ExternalOutput").ap()

    from contextlib import ExitStack

    with tile.TileContext(nc) as tc, ExitStack() as ctx:
        cp = ctx.enter_context(tc.tile_pool(name="const", bufs=1))
        dram = ctx.enter_context(tc.tile_pool(name="dram", bufs=1, space="DRAM"))
        lhsp = ctx.enter_context(tc.tile_pool(name="lhsp", bufs=4))
        kvtp = ctx.enter_context(tc.tile_pool(name="kvtp", bufs=4))
        qtp = ctx.enter_context(tc.tile_pool(name="qtp", bufs=4))
        qtbp = ctx.enter_context(tc.tile_pool(name="qtbp", bufs=3))
        kvgp = ctx.enter_context(tc.tile_pool(name="kvgp", bufs=12))
        shp = ctx.enter_context(tc.tile_pool(name="shp", bufs=12))
        scrp = ctx.enter_context(tc.tile_pool(name="scrp", bufs=12))
        smp = ctx.enter_context(tc.tile_pool(name="smp", bufs=12))
        msop = ctx.enter_context(tc.tile_pool(name="msop", bufs=8))
        hop = ctx.enter_context(tc.tile_pool(name="hop", bufs=4))
        htp = ctx.enter_context(tc.tile_pool(name="htp", bufs=1))
        atp = ctx.enter_context(tc.tile_pool(name="atp", bufs=1))
        bigp = ctx.enter_context(tc.tile_pool(name="bigp", bufs=1))
        gpp = ctx.enter_context(tc.tile_pool(name="gpp", bufs=2, space="PSUM"))
        segp = ctx.enter_context(tc.tile_pool(name="segp", bufs=2, space="PSUM"))
        sump = ctx.enter_context(tc.tile_pool(name="sump", bufs=2, space="PSUM"))

        def pj_tile():
            return gpp.tile([128, 264], f32, name="pj", tag="gp")

        def evict(out_ap, ps_ap, bias_tile):
            if zero_bias:
                nc.scalar.copy(out=out_ap, in_=ps_ap)
            else:
                nc.vector.tensor_add(out=out_ap, in0=ps_ap, in1=bias_tile[:])

        # ---- DRAM scratch ----
        KV1 = dram.tile([NFULL, 512], bf16, name="KV1")
        Q1 = dram.tile([NSPAD, 264], bf16, name="Q1")
        MS1 = dram.tile([NSPAD, 256], f32, name="MS1")
        MSH1 = [dram.tile([NSPAD, 2], f32, name=f"MSH1{h}") for h in range(2)]
        KV2S = dram.tile([NS, 512], bf16, name="KV2S")
        KV2F = dram.tile([N, 512], bf16, name="KV2F", addr_space="Shared")
        Q2 = dram.tile([NSPAD, 264], bf16, name="Q2")
        MS2 = dram.tile([NSPAD, 256], f32, name="MS2")
        MSH2 = [dram.tile([NSPAD, 2], f32, name=f"MSH2{h}") for h in range(2)]
        STI1 = dram.tile([128, 4], f32, name="STI1")
        STO1 = dram.tile([128 * R, 4], f32, name="STO1", addr_space="Shared")
        STI2 = dram.tile([128, 4], f32, name="STI2")
        STO2 = dram.tile([128 * R, 4], f32, name="STO2", addr_space="Shared")

        # ---- resident SBUF constants ----
        def load(name, ap, shape, dtype=f32):
            t = cp.tile(shape, dtype, name=name)
            nc.sync.dma_start(t[:], ap[:, :])
            return t

        xts = load("xts", XTS, [128, NSPAD], bf16)
        srcs = load("srcs", SRC, [128, T], i32)
        eavs = load("eavs", EAV, [128, T])
        sens = load("sens", SEN, [128, T * BLK], bf16)
        snes = load("snes", SNE, [128, T * 128], bf16)
        oseas = load("oseas", OSEA, [128, 2 * T], bf16)
        w1 = {k: load(k.lower(), v, [128, 256], bf16) for k, v in W1.items()}
        b1 = {k: load(k.lower(), v, [128, 256]) for k, v in B1.items()}
        we1 = load("we1", WE1, [128, 256])
        w2 = {k: load(k.lower(), v, [128, 512], bf16) for k, v in W2.items()}
        b2 = {k: load(k.lower(), v, [128, 256]) for k, v in B2.items()}
        we2 = load("we2", WE2, [128, 256])
        g1t = load("g1t", G1T, [128, 2])
        b1tt = load("b1tt", B1TT, [128, 2])
        g2t = load("g2t", G2T, [128, 2])
        b2tt = load("b2tt", B2TT, [128, 2])
        ident = cp.tile([128, 128], f32, name="ident")
        make_identity(nc, ident[:])

        ht = [htp.tile([128, NSPAD], f32, name=f"ht{h}") for h in range(2)]

        # ================= layer-1 projections =================
        # K/V for ALL nodes, replicated on every core.
        nt = 0
        while nt < NT1:
            nj = min(4, NT1 - nt)
            lhs = lhsp.tile([128, 4 * 128], bf16, name="lhs")
            nc.sync.dma_start(lhs[:, 0:nj * 128], XTF[:, nt * 128:(nt + nj) * 128])
            kvt = kvtp.tile([128, 4 * 512], bf16, name="kvt")
            for j in range(nj):
                for Wn, Bn, off in (("WK1", "BK1", 0), ("WV1", "BV1", 256)):
                    ps = pj_tile()[:, 0:256]
                    nc.tensor.matmul(ps[:], lhsT=lhs[:, j * 128:(j + 1) * 128],
                                     rhs=w1[Wn][:], start=True, stop=True)
                    evict(kvt[:, j * 512 + off:j * 512 + off + 256], ps[:], b1[Bn])
            nc.sync.dma_start(
                KV1[nt * 128:(nt + nj) * 128, :].rearrange("(j p) c -> p j c", p=128),
                kvt[:, 0:nj * 512].rearrange("p (j c) -> p j c", c=512))
            nt += nj

        def q_proj(lhs_chunks, wq, bq, we, Qd):
            for mt in range(NTS):
                sl = slice(mt * 128, (mt + 1) * 128)
                qt = qtp.tile([128, 264], bf16, name="qt")
                ps = pj_tile()[:, 0:256]
                nkc = len(lhs_chunks)
                for kc in range(nkc):
                    nc.tensor.matmul(ps[:], lhsT=lhs_chunks[kc][:, sl],
                                     rhs=wq[:, kc * 256:(kc + 1) * 256],
                                     start=(kc == 0), stop=(kc == nkc - 1))
                evict(qt[:, 0:256], ps[:], bq)
                nc.vector.memset(qt[:, 256:264], 0.0)
                scr = scrp.tile([128, 256], f32, name="scr")
                nc.vector.tensor_mul(out=scr[:], in0=qt[:, 0:256], in1=we[:, 0:256])
                tsum = smp.tile([128, 2], f32, name="tsum")
                nc.vector.reduce_sum(out=tsum[:],
                                     in_=scr[:].rearrange("p (h c) -> p h c", c=128),
                                     axis=AX.X)
                nc.vector.tensor_copy(out=qt[:, 256:258], in_=tsum[:])
                nc.sync.dma_start(Qd[sl, :], qt[:])

        q_proj([xts], w1["WQ1"], b1["BQ1"], we1, Q1)

        def bail():
            z = bigp.tile([128, NSPAD], f32, name="sq")
            nc.vector.memset(z[:], 0.0)
            for half in range(2):
                nc.sync.dma_start(OUTT[half * 128:(half + 1) * 128, :], z[:])

        # ================= edge phase =================
        def edge_phase(KVt, Qd, MSt, MSHt):
            kv_rows = KVt.shape[0]
            ti = 0
            for b in range(NB):
                msv = [segp.tile([BLK, 128], f32, name=f"msv{h}") for h in range(2)]
                mss = sump.tile([128, 2], f32, name="mss")
                qtb = qtbp.tile([128, 264], bf16, name="qtb")
                nc.vector.memset(qtb[BLK:128, :], 0.0)
                nc.sync.dma_start(qtb[0:BLK, :], Qd[b * BLK:(b + 1) * BLK, :])
                last = int(n_b[b]) - 1
                for i in range(int(n_b[b])):
                    t = ti
                    ti += 1
                    kvg = kvgp.tile([128, 512], bf16, name="kvg")
                    nc.gpsimd.indirect_dma_start(
                        out=kvg[:], out_offset=None, in_=KVt[:, :],
                        in_offset=bass.IndirectOffsetOnAxis(ap=srcs[:, t:t + 1], axis=0),
                        bounds_check=kv_rows - 1, oob_is_err=False)
                    qgps = gpp.tile([128, 264], f32, name="qgps", tag="gp")
                    nc.tensor.matmul(out=qgps[:, 0:258],
                                     lhsT=snes[:, t * 128:(t + 1) * 128],
                                     rhs=qtb[:, 0:258], start=True, stop=True)
                    scr = scrp.tile([128, 256], f32, name="scr")
                    asum = smp.tile([128, 2], f32, name="asum")
                    nc.vector.tensor_mul(out=scr[:], in0=qgps[:, 0:256],
                                         in1=kvg[:, 0:256])
                    nc.vector.reduce_sum(out=asum[:],
                                         in_=scr[:].rearrange("p (h c) -> p h c", c=128),
                                         axis=AX.X)
                    eat = smp.tile([128, 2], f32, name="eat")
                    nc.vector.tensor_mul(out=eat[:], in0=qgps[:, 256:258],
                                         in1=eavs[:, t:t + 1].to_broadcast([128, 2]))
                    nc.vector.tensor_add(out=eat[:], in0=eat[:], in1=asum[:])
                    ex = smp.tile([128, 2], f32, name="ex")
                    nc.scalar.activation(out=ex[:], in_=eat[:], func=Act.Exp)
                    sh = shp.tile([128, 128], bf16, name="sh")
                    for h in range(2):
                        nc.scalar.activation(out=sh[:, h * BLK:(h + 1) * BLK],
                                             in_=sens[:, t * BLK:(t + 1) * BLK],
                                             func=Act.Copy, scale=ex[:, h:h + 1])
                    for h in range(2):
                        nc.tensor.matmul(out=msv[h][:, 0:128],
                                         lhsT=sh[:, h * BLK:(h + 1) * BLK],
                                         rhs=kvg[:, 256 + h * 128:256 + (h + 1) * 128],
                                         start=(i == 0), stop=(i == last))
                    nc.tensor.matmul(out=mss[:, 0:2], lhsT=sh[:, 0:128],
                                     rhs=oseas[:, 2 * t:2 * t + 2],
                                     start=(i == 0), stop=(i == last))
                mso = msop.tile([BLK, 256], f32, name="mso")
                for h in range(2):
                    nc.any.tensor_copy(out=mso[:, h * 128:(h + 1) * 128],
                                       in_=msv[h][:, 0:128])
                msos = msop.tile([128, 2], f32, name="msos")
                nc.any.tensor_copy(out=msos[:], in_=mss[:])
                nc.sync.dma_start(MSt[b * BLK:(b + 1) * BLK, :], mso[:])
                for h in range(2):
                    nc.sync.dma_start(MSHt[h][b * BLK:(b + 1) * BLK, :],
                                      msos[h * BLK:(h + 1) * BLK, :])
            assert ti == T

        if stage >= 2:
            edge_phase(KV1, Q1, MS1, MSH1)

        # ================= node phase =================
        def node_phase(MSt, MSHt, lhs_chunks, ws, bs, we, gt, bt, at_names, STI, STO, at_dt):
            nkc = len(lhs_chunks)
            for mt in range(NTS):
                sl = slice(mt * 128, (mt + 1) * 128)
                ms = msop.tile([128, 264], f32, name="msi")
                nc.sync.dma_start(ms[:, 0:256], MSt[sl, :])
                for h in range(2):
                    nc.sync.dma_start(ms[:, 256 + 2 * h:258 + 2 * h], MSHt[h][sl, :])
                ps = pj_tile()[:, 0:256]
                for kc in range(nkc):
                    nc.tensor.matmul(ps[:], lhsT=lhs_chunks[kc][:, sl],
                                     rhs=ws[:, kc * 256:(kc + 1) * 256],
                                     start=(kc == 0), stop=(kc == nkc - 1))
                den = smp.tile([128, 2], f32, name="den")
                for h in range(2):
                    nc.vector.tensor_scalar_add(out=den[:, h:h + 1],
                                                in0=ms[:, 256 + 2 * h:257 + 2 * h],
                                                scalar1=1e-16)
                rec = smp.tile([128, 2], f32, name="rec")
                nc.vector.reciprocal(out=rec[:], in_=den[:])
                tmp = scrp.tile([128, 256], f32, name="scr")
                for h in range(2):
                    hs = slice(h * 128, (h + 1) * 128)
                    nc.scalar.activation(out=tmp[:, hs], in_=we[:, hs],
                                         func=Act.Copy,
                                         scale=ms[:, 257 + 2 * h:258 + 2 * h])
                nc.vector.tensor_add(out=tmp[:], in0=tmp[:], in1=ms[:, 0:256])
                ho = hop.tile([128, 256], f32, name="ho")
                for h in range(2):
                    hs = slice(h * 128, (h + 1) * 128)
                    nc.scalar.activation(out=ho[:, hs], in_=tmp[:, hs],
                                         func=Act.Copy, scale=rec[:, h:h + 1])
                nc.vector.tensor_add(out=ho[:], in0=ho[:], in1=ps[:])
                if not zero_bias:
                    nc.vector.tensor_add(out=ho[:], in0=ho[:], in1=bs[:])
                for half in range(2):
                    tp = gpp.tile([128, 128], f32, name="tp", tag="gp")
                    nc.tensor.transpose(out=tp[:], in_=ho[:, half * 128:(half + 1) * 128],
                                        identity=ident[:])
                    nc.vector.tensor_copy(out=ht[half][:, sl], in_=tp[:])
            stt = smp.tile([128, 4], f32, name="stt")
            for half in range(2):
                nc.vector.reduce_sum(out=stt[:, half:half + 1], in_=ht[half][:, 0:NS],
                                     axis=AX.X)
                sq = bigp.tile([128, NSPAD], f32, name="sq")
                nc.scalar.activation(out=sq[:, 0:NS], in_=ht[half][:, 0:NS],
                                     func=Act.Square,
                                     accum_out=stt[:, 2 + half:3 + half])
            nc.sync.dma_start(STI[:, :], stt[:])
            nc.gpsimd.collective_compute(
                "AllGather", Alu.bypass, replica_groups=[list(range(R))],
                ins=[STI[:].opt()], outs=[STO[:].opt()])
            gsta = smp.tile([128, 4, R], f32, name="gsta")
            nc.sync.dma_start(
                gsta[:], STO[:].rearrange("(r p) k -> p k r", p=128))
            gst = smp.tile([128, 4], f32, name="gst")
            nc.vector.reduce_sum(out=gst[:], in_=gsta[:], axis=AX.X)
            mean = smp.tile([128, 2], f32, name="mean")
            nc.vector.tensor_scalar_mul(out=mean[:], in0=gst[:, 0:2], scalar1=1.0 / N)
            var = smp.tile([128, 2], f32, name="var")
            nc.vector.tensor_scalar_mul(out=var[:], in0=gst[:, 2:4], scalar1=1.0 / N)
            m2 = smp.tile([128, 2], f32, name="m2")
            nc.vector.tensor_mul(out=m2[:], in0=mean[:], in1=mean[:])
            nc.vector.tensor_sub(out=var[:], in0=var[:], in1=m2[:])
            nc.vector.tensor_scalar_add(out=var[:], in0=var[:], scalar1=EPS)
            sd = smp.tile([128, 2], f32, name="sd")
            nc.scalar.activation(out=sd[:], in_=var[:], func=Act.Sqrt)
            rsd = smp.tile([128, 2], f32, name="rsd")
            nc.vector.reciprocal(out=rsd[:], in_=sd[:])
            sc2 = smp.tile([128, 2], f32, name="sc2")
            nc.vector.tensor_mul(out=sc2[:], in0=gt[:], in1=rsd[:])
            sh2 = smp.tile([128, 2], f32, name="sh2")
            nc.vector.tensor_mul(out=sh2[:], in0=mean[:], in1=sc2[:])
            nc.vector.tensor_sub(out=sh2[:], in0=bt[:], in1=sh2[:])
            at = [atp.tile([128, NSPAD], at_dt, name=nm) for nm in at_names]
            for half in range(2):
                nc.scalar.activation(out=at[half][:], in_=ht[half][:], func=Act.Relu,
                                     scale=sc2[:, half:half + 1],
                                     bias=sh2[:, half:half + 1])
            return at

        if stage >= 3:
            at1 = node_phase(MS1, MSH1, [xts], w1["WS1"], b1["BS1"], we1,
                             g1t, b1tt, ["at10", "at11"], STI1, STO1, bf16)

        # ================= layer-2 projections (data-parallel + AllGather) ====
        if stage >= 4:
            for mt in range(NTS):
                sl = slice(mt * 128, (mt + 1) * 128)
                kvt = kvtp.tile([128, 512], bf16, name="kvt")
                for Wn, Bn, off in (("WK2", "BK2", 0), ("WV2", "BV2", 256)):
                    ps = pj_tile()[:, 0:256]
                    for kc in range(2):
                        nc.tensor.matmul(ps[:], lhsT=at1[kc][:, sl],
                                         rhs=w2[Wn][:, kc * 256:(kc + 1) * 256],
                                         start=(kc == 0), stop=(kc == 1))
                    evict(kvt[:, off:off + 256], ps[:], b2[Bn])
                if mt < NTS - 1:
                    nc.sync.dma_start(KV2S[sl, :], kvt[:])
                else:
                    nc.sync.dma_start(KV2S[mt * 128:NS, :], kvt[:NS - mt * 128, :])
            nc.gpsimd.collective_compute(
                "AllGather", mybir.AluOpType.bypass, replica_groups=[list(range(R))],
                ins=[KV2S[:].opt()], outs=[KV2F[:].opt()])

            q_proj(at1, w2["WQ2"], b2["BQ2"], we2, Q2)

        if stage >= 5:
            edge_phase(KV2F, Q2, MS2, MSH2)

        if stage >= 99:
            at2 = node_phase(MS2, MSH2, at1, w2["WS2"], b2["BS2"], we2,
                             g2t, b2tt, ["at20", "at21"], STI2, STO2, f32)
            for half in range(2):
                nc.sync.dma_start(OUTT[half * 128:(half + 1) * 128, :], at2[half][:])
        else:
            bail()

    nc.compile()
    return nc


def _pack_inputs(inputs, per_core):
    import ml_dtypes
    bfh = ml_dtypes.bfloat16
    x = np.asarray(inputs["x"], np.float32)
    xT = np.zeros((128, NFULL), np.float32)
    xT[:, :N] = x.T
    common = {"XTF": np.ascontiguousarray(xT).astype(bfh)}

    def bc(v):
        return np.ascontiguousarray(np.broadcast_to(
            np.asarray(v, np.float32).reshape(1, -1), (128, v.shape[-1])))

    def chunks2(w):
        w = np.asarray(w, np.float32)
        return np.ascontiguousarray(np.concatenate([w[0:128], w[128:256]], axis=1))

    for L, nm in ((1, "1"), (2, "2")):
        for key, wn in (("Wk", "WK"), ("Wv", "WV"), ("Wq", "WQ"), ("Ws", "WS")):
            w = np.asarray(inputs[key + nm], np.float32)
            if wn == "WQ":
                w = w * np.float32(INV)
            if L == 1:
                common[wn + nm] = np.ascontiguousarray(w).astype(bfh)
            else:
                common[wn + nm] = chunks2(w).astype(bfh)
        for key, bn in (("bk", "BK"), ("bv", "BV"), ("bq", "BQ"), ("bs", "BS")):
            b = np.asarray(inputs[key + nm], np.float32)
            if bn == "BQ":
                b = b * np.float32(INV)
            common[bn + nm] = bc(b)
        common["WE" + nm] = bc(np.asarray(inputs["We" + nm], np.float32).reshape(-1))
        common["G" + nm + "T"] = np.ascontiguousarray(
            np.asarray(inputs["g" + nm], np.float32).reshape(2, 128).T)
        common["B" + nm + "TT"] = np.ascontiguousarray(
            np.asarray(inputs["b" + nm], np.float32).reshape(2, 128).T)

    in_maps = []
    for r in range(R):
        m = dict(common)
        xs = np.zeros((128, NSPAD), np.float32)
        xs[:, :NS] = x[NS * r:NS * (r + 1)].T
        m["XTS"] = np.ascontiguousarray(xs).astype(bfh)
        m.update(per_core[r])
        in_maps.append(m)
    return in_maps


def kernel(**inputs):
    from concourse import bass_utils

    edge_index = np.asarray(inputs["edge_index"])
    edge_attr = np.asarray(inputs["edge_attr"], np.float32)
    zb = all(not np.any(np.asarray(inputs[k]))
             for k in ("bq1", "bk1", "bv1", "bs1", "bq2", "bk2", "bv2", "bs2"))
    key = (hash(edge_index.tobytes()), zb)
    if key not in _CACHE:
        per_core, n_b, T = _prepare(edge_index, edge_attr)
        nc = _build(T, n_b, zero_bias=zb)
        _CACHE[key] = (nc, per_core)
    nc, per_core = _CACHE[key]

    in_maps = _pack_inputs(inputs, per_core)
    import os
    trace = bool(int(os.environ.get("KBENCH_TRACE", "0")))
    res = bass_utils.run_bass_kernel_spmd(
        nc, in_maps, core_ids=list(range(R)), trace=trace)
    kernel.last_result = res
    out = np.concatenate(
        [res.results[r]["OUTT"][:, :NS].T for r in range(R)], axis=0)
    return np.ascontiguousarray(out)


# revision 44
# speedup vs baseline: 1.0194x; 1.0194x over previous
"""Trainium2 Bass kernel for the 2-layer TransformerConv GNN (edge-parallel, 8 cores).

Strategy (edge parallel, per sharding hint):
  - Sort edges by dst; shard nodes into 8 equal slices of 1250; each core owns
    all edges whose dst falls in its slice, so segment-softmax and scatter-add
    are core-local (no softmax-stat collectives needed).
  - Layer-1 K/V node projections are computed replicated on every core
    (input x is replicated); layer-2 K/V are data-parallel over nodes followed
    by an AllGather of the fused K|V table. BN statistics use a tiny AllReduce.
  - Per core, edges are grouped into 64-node blocks; each block's <=9 tiles of
    128 edges accumulate their segment-sums in PSUM via a 0/1 selection-matrix
    matmul, so no indirect scatter is needed.
  - Per 128-edge tile: K|V rows are gathered with one indirect DMA from the
    fused [N,512] table, q rows (+ precomputed t = (q . We)/sqrt(C)) with a
    second indirect DMA. Softmax skips the max-subtraction (logits are O(1))
    and normalization is deferred to the node phase:
       out_n = (sum_e ex_e * v_e + (sum_e ex_e*ea_e) * We) / (sum_e ex_e + 1e-16)
"""

import math

import numpy as np

N, E, H, C, D, F = 10000, 160000, 2, 128, 256, 128
R, NS, BLK = 8, 1250, 64
NB = (NS + BLK - 1) // BLK          # 20 blocks/core
NSPAD = NB * BLK                    # 1280
NFULL = 10112                       # 79*128, padded global node count
NT1 = NFULL // 128                  # 79
NTS = NSPAD // 128                  # 10
EPS = 1e-5
INV = 1.0 / math.sqrt(C)

_CACHE = {}


def _prepare(edge_index, edge_attr):
    """Host-side index preprocessing -> per-core tile arrays (data only)."""
    src = edge_index[0].astype(np.int32)
    dst = edge_index[1].astype(np.int32)
    ea = edge_attr[:, 0].astype(np.float32)
    perm = np.argsort(dst, kind="stable")
    sdst, ssrc, sea = dst[perm], src[perm], ea[perm]
    bounds = np.searchsorted(sdst, np.arange(0, N + 1, NS))
    core_info = []
    cnts = np.zeros((R, NB), dtype=np.int64)
    for r in range(R):
        lo, hi = bounds[r], bounds[r + 1]
        ldst = sdst[lo:hi] - NS * r
        bb = np.searchsorted(ldst, np.arange(0, NSPAD + 1, BLK))
        cnts[r] = np.diff(bb)
        core_info.append((lo, bb))
    n_b = np.maximum(1, np.ceil(cnts.max(axis=0) / 128).astype(np.int64))
    T = int(n_b.sum())
    per_core = []
    for r in range(R):
        lo, bb = core_info[r]
        SRC = np.zeros((T, 128), np.int32)
        DST = np.zeros((T, 128), np.int32)
        EAV = np.zeros((T, 128), np.float32)
        S = np.zeros((T, 128, BLK), np.float32)
        t = 0
        for b in range(NB):
            base = lo + bb[b]
            nb_edges = bb[b + 1] - bb[b]
            for i in range(n_b[b]):
                e0 = min(128 * i, nb_edges)
                e1 = min(128 * (i + 1), nb_edges)
                cnt = e1 - e0
                if cnt > 0:
                    sl = slice(base + e0, base + e1)
                    ld = sdst[sl] - NS * r
                    SRC[t, :cnt] = ssrc[sl]
                    DST[t, :cnt] = ld
                    EAV[t, :cnt] = sea[sl]
                    S[t, np.arange(cnt), ld - BLK * b] = 1.0
                DST[t, cnt:] = BLK * b
                t += 1
        import ml_dtypes
        bf16 = ml_dtypes.bfloat16
        Sne = np.zeros((T, 128, 128), np.float32)
        Sne[:, :BLK, :] = S.transpose(0, 2, 1)
        onesea = np.zeros((128, 2 * T), np.float32)
        onesea[:, 0::2] = 1.0
        onesea[:, 1::2] = EAV.T
        per_core.append(
            dict(
                SRC=np.ascontiguousarray(SRC.T),                       # [128, T]
                EAV=np.ascontiguousarray(EAV.T),                       # [128, T]
                SEN=np.ascontiguousarray(
                    S.transpose(1, 0, 2).reshape(128, T * BLK)).astype(bf16),
                SNE=np.ascontiguousarray(
                    Sne.transpose(1, 0, 2).reshape(128, T * 128)).astype(bf16),
                OSEA=np.ascontiguousarray(onesea).astype(bf16),
            )
        )
    return per_core, n_b, T


def _build(T, n_b, stage=99, zero_bias=False):
    """Build + schedule the (shared-across-cores) Bass program.

    stage: 1=L1 projections, 2=+edge1, 3=+node1 (AllReduce), 4=+proj2 (AllGather),
           5=+edge2, 99=full.
    """
    import concourse.bass as bass
    import concourse.mybir as mybir
    import concourse.tile as tile
    from concourse import bacc
    from concourse.masks import make_identity

    f32 = mybir.dt.float32
    bf16 = mybir.dt.bfloat16
    i32 = mybir.dt.int32
    Alu = mybir.AluOpType
    Act = mybir.ActivationFunctionType
    AX = mybir.AxisListType

    nc = bacc.Bacc("TRN2", target_bir_lowering=False, debug=False,
                   enable_asserts=False, num_devices=R)

    def din(name, shape, dtype=f32):
        return nc.dram_tensor(name, shape, dtype, kind="ExternalInput").ap()

    XTF = din("XTF", [128, NFULL], bf16)
    XTS = din("XTS", [128, NSPAD], bf16)
    SRC = din("SRC", [128, T], i32)
    EAV = din("EAV", [128, T])
    SEN = din("SEN", [128, T * BLK], bf16)
    SNE = din("SNE", [128, T * 128], bf16)
    OSEA = din("OSEA", [128, 2 * T], bf16)
    W1 = {k: din(k, [128, 256], bf16) for k in ("WK1", "WV1", "WQ1", "WS1")}
    B1 = {k: din(k, [128, 256]) for k in ("BK1", "BV1", "BQ1", "BS1")}
    WE1 = din("WE1", [128, 256])
    W2 = {k: din(k, [128, 512], bf16) for k in ("WK2", "WV2", "WQ2", "WS2")}
    B2 = {k: din(k, [128, 256]) for k in ("BK2", "BV2", "BQ2", "BS2")}
    WE2 = din("WE2", [128, 256])
    G1T, B1TT = din("G1T", [128, 2]), din("B1TT", [128, 2])
    G2T, B2TT = din("G2T", [128, 2]), din("B2TT", [128, 2])
    OUTT = nc.dram_tensor("OUTT", [256, NSPAD], f32, kind="ExternalOutput").ap()

    from contextlib import ExitStack

    with tile.TileContext(nc) as tc, ExitStack() as ctx:
        cp = ctx.enter_context(tc.tile_pool(name="const", bufs=1))
        dram = ctx.enter_context(tc.tile_pool(name="dram", bufs=1, space="DRAM"))
        lhsp = ctx.enter_context(tc.tile_pool(name="lhsp", bufs=4))
        kvtp = ctx.enter_context(tc.tile_pool(name="kvtp", bufs=4))
        qtp = ctx.enter_context(tc.tile_pool(name="qtp", bufs=4))
        qtbp = ctx.enter_context(tc.tile_pool(name="qtbp", bufs=3))
        kvgp = ctx.enter_context(tc.tile_pool(name="kvgp", bufs=12))
        shp = ctx.enter_context(tc.tile_pool(name="shp", bufs=12))
        scrp = ctx.enter_context(tc.tile_pool(name="scrp", bufs=12))
        smp = ctx.enter_context(tc.tile_pool(name="smp", bufs=12))
        msop = ctx.enter_context(tc.tile_pool(name="msop", bufs=8))
        hop = ctx.enter_context(tc.tile_pool(name="hop", bufs=4))
        htp = ctx.enter_context(tc.tile_pool(name="htp", bufs=1))
        atp = ctx.enter_context(tc.tile_pool(name="atp", bufs=1))
        bigp = ctx.enter_context(tc.tile_pool(name="bigp", bufs=1))
        gpp = ctx.enter_context(tc.tile_pool(name="gpp", bufs=2, space="PSUM"))
        segp = ctx.enter_context(tc.tile_pool(name="segp", bufs=2, space="PSUM"))
        sump = ctx.enter_context(tc.tile_pool(name="sump", bufs=2, space="PSUM"))

        def pj_tile():
            return gpp.tile([128, 264], f32, name="pj", tag="gp")

        def evict(out_ap, ps_ap, bias_tile):
            if zero_bias:
                nc.scalar.copy(out=out_ap, in_=ps_ap)
            else:
                nc.vector.tensor_add(out=out_ap, in0=ps_ap, in1=bias_tile[:])

        # ---- DRAM scratch ----
        KV1 = dram.tile([NFULL, 512], bf16, name="KV1")
        Q1 = dram.tile([NSPAD, 264], bf16, name="Q1")
        MS1 = dram.tile([NSPAD, 256], f32, name="MS1")
        MSH1 = [dram.tile([NSPAD, 2], f32, name=f"MSH1{h}") for h in range(2)]
        KV2S = dram.tile([NS, 512], bf16, name="KV2S")
        KV2F = dram.tile([N, 512], bf16, name="KV2F", addr_space="Shared")
        Q2 = dram.tile([NSPAD, 264], bf16, name="Q2")
        MS2 = dram.tile([NSPAD, 256], f32, name="MS2")
        MSH2 = [dram.tile([NSPAD, 2], f32, name=f"MSH2{h}") for h in range(2)]
        STI1 = dram.tile([128, 4], f32, name="STI1")
        STO1 = dram.tile([128 * R, 4], f32, name="STO1", addr_space="Shared")
        STI2 = dram.tile([128, 4], f32, name="STI2")
        STO2 = dram.tile([128 * R, 4], f32, name="STO2", addr_space="Shared")

        # ---- resident SBUF constants ----
        def load(name, ap, shape, dtype=f32):
            t = cp.tile(shape, dtype, name=name)
            nc.sync.dma_start(t[:], ap[:, :])
            return t

        xts = load("xts", XTS, [128, NSPAD], bf16)
        w1 = {k: load(k.lower(), v, [128, 256], bf16) for k, v in W1.items()}
        b1 = {k: load(k.lower(), v, [128, 256]) for k, v in B1.items()}
        we1 = load("we1", WE1, [128, 256])
        w2 = {k: load(k.lower(), v, [128, 512], bf16) for k, v in W2.items()}
        b2 = {k: load(k.lower(), v, [128, 256]) for k, v in B2.items()}
        we2 = load("we2", WE2, [128, 256])
        g1t = load("g1t", G1T, [128, 2])
        b1tt = load("b1tt", B1TT, [128, 2])
        g2t = load("g2t", G2T, [128, 2])
        b2tt = load("b2tt", B2TT, [128, 2])
        ident = cp.tile([128, 128], f32, name="ident")
        make_identity(nc, ident[:])

        ht = [htp.tile([128, NSPAD], f32, name=f"ht{h}") for h in range(2)]

        # ================= layer-1 projections =================
        # K/V for ALL nodes, replicated on every core.
        nt = 0
        lhs = None
        while nt < NT1:
            if nt % 8 == 0:
                nl = min(8, NT1 - nt)
                lhs = lhsp.tile([128, 8 * 128], bf16, name="lhs")
                nc.sync.dma_start(lhs[:, 0:nl * 128],
                                  XTF[:, nt * 128:(nt + nl) * 128])
                lbase = nt
            nj = min(4, NT1 - nt)
            kvt = kvtp.tile([128, 4 * 512], bf16, name="kvt")
            for j in range(nj):
                jj = nt - lbase + j
                for Wn, Bn, off in (("WK1", "BK1", 0), ("WV1", "BV1", 256)):
                    ps = pj_tile()[:, 0:256]
                    nc.tensor.matmul(ps[:], lhsT=lhs[:, jj * 128:(jj + 1) * 128],
                                     rhs=w1[Wn][:], start=True, stop=True)
                    evict(kvt[:, j * 512 + off:j * 512 + off + 256], ps[:], b1[Bn])
            nc.sync.dma_start(
                KV1[nt * 128:(nt + nj) * 128, :].rearrange("(j p) c -> p j c", p=128),
                kvt[:, 0:nj * 512].rearrange("p (j c) -> p j c", c=512))
            nt += nj

        def q_proj(lhs_chunks, wq, bq, we, Qd):
            for mt in range(NTS):
                sl = slice(mt * 128, (mt + 1) * 128)
                qt = qtp.tile([128, 264], bf16, name="qt")
                ps = pj_tile()[:, 0:256]
                nkc = len(lhs_chunks)
                for kc in range(nkc):
                    nc.tensor.matmul(ps[:], lhsT=lhs_chunks[kc][:, sl],
                                     rhs=wq[:, kc * 256:(kc + 1) * 256],
                                     start=(kc == 0), stop=(kc == nkc - 1))
                evict(qt[:, 0:256], ps[:], bq)
                nc.vector.memset(qt[:, 256:264], 0.0)
                scr = scrp.tile([128, 256], f32, name="scr")
                nc.vector.tensor_mul(out=scr[:], in0=qt[:, 0:256], in1=we[:, 0:256])
                tsum = smp.tile([128, 2], f32, name="tsum")
                nc.vector.reduce_sum(out=tsum[:],
                                     in_=scr[:].rearrange("p (h c) -> p h c", c=128),
                                     axis=AX.X)
                nc.vector.tensor_copy(out=qt[:, 256:258], in_=tsum[:])
                nc.sync.dma_start(Qd[sl, :], qt[:])

        q_proj([xts], w1["WQ1"], b1["BQ1"], we1, Q1)

        # edge-phase constants: loaded after projections are queued so the
        # projection operand DMAs aren't stuck behind these ~18MB transfers
        srcs = load("srcs", SRC, [128, T], i32)
        eavs = load("eavs", EAV, [128, T])
        sens = load("sens", SEN, [128, T * BLK], bf16)
        snes = load("snes", SNE, [128, T * 128], bf16)
        oseas = load("oseas", OSEA, [128, 2 * T], bf16)

        def bail():
            z = bigp.tile([128, NSPAD], f32, name="sq")
            nc.vector.memset(z[:], 0.0)
            for half in range(2):
                nc.sync.dma_start(OUTT[half * 128:(half + 1) * 128, :], z[:])

        # ================= edge phase =================
        def edge_phase(KVt, Qd, MSt, MSHt):
            kv_rows = KVt.shape[0]
            ti = 0
            for b in range(NB):
                msv = [segp.tile([BLK, 128], f32, name=f"msv{h}") for h in range(2)]
                mss = sump.tile([128, 2], f32, name="mss")
                qtb = qtbp.tile([128, 264], bf16, name="qtb")
                nc.vector.memset(qtb[BLK:128, :], 0.0)
                nc.sync.dma_start(qtb[0:BLK, :], Qd[b * BLK:(b + 1) * BLK, :])
                last = int(n_b[b]) - 1
                for i in range(int(n_b[b])):
                    t = ti
                    ti += 1
                    kvg = kvgp.tile([128, 512], bf16, name="kvg")
                    nc.gpsimd.indirect_dma_start(
                        out=kvg[:], out_offset=None, in_=KVt[:, :],
                        in_offset=bass.IndirectOffsetOnAxis(ap=srcs[:, t:t + 1], axis=0),
                        bounds_check=kv_rows - 1, oob_is_err=False)
                    qgps = gpp.tile([128, 264], f32, name="qgps", tag="gp")
                    nc.tensor.matmul(out=qgps[:, 0:258],
                                     lhsT=snes[:, t * 128:(t + 1) * 128],
                                     rhs=qtb[:, 0:258], start=True, stop=True)
                    scr = scrp.tile([128, 256], f32, name="scr")
                    asum = smp.tile([128, 2], f32, name="asum")
                    nc.vector.tensor_mul(out=scr[:], in0=qgps[:, 0:256],
                                         in1=kvg[:, 0:256])
                    nc.vector.reduce_sum(out=asum[:],
                                         in_=scr[:].rearrange("p (h c) -> p h c", c=128),
                                         axis=AX.X)
                    eat = smp.tile([128, 2], f32, name="eat")
                    nc.vector.tensor_mul(out=eat[:], in0=qgps[:, 256:258],
                                         in1=eavs[:, t:t + 1].to_broadcast([128, 2]))
                    nc.vector.tensor_add(out=eat[:], in0=eat[:], in1=asum[:])
                    ex = smp.tile([128, 2], f32, name="ex")
                    nc.scalar.activation(out=ex[:], in_=eat[:], func=Act.Exp)
                    sh = shp.tile([128, 128], bf16, name="sh")
                    for h in range(2):
                        nc.scalar.activation(out=sh[:, h * BLK:(h + 1) * BLK],
                                             in_=sens[:, t * BLK:(t + 1) * BLK],
                                             func=Act.Copy, scale=ex[:, h:h + 1])
                    for h in range(2):
                        nc.tensor.matmul(out=msv[h][:, 0:128],
                                         lhsT=sh[:, h * BLK:(h + 1) * BLK],
                                         rhs=kvg[:, 256 + h * 128:256 + (h + 1) * 128],
                                         start=(i == 0), stop=(i == last))
                    nc.tensor.matmul(out=mss[:, 0:2], lhsT=sh[:, 0:128],
                                     rhs=oseas[:, 2 * t:2 * t + 2],
                                     start=(i == 0), stop=(i == last))
                mso = msop.tile([BLK, 256], f32, name="mso")
                for h in range(2):
                    nc.any.tensor_copy(out=mso[:, h * 128:(h + 1) * 128],
                                       in_=msv[h][:, 0:128])
                msos = msop.tile([128, 2], f32, name="msos")
                nc.any.tensor_copy(out=msos[:], in_=mss[:])
                nc.sync.dma_start(MSt[b * BLK:(b + 1) * BLK, :], mso[:])
                for h in range(2):
                    nc.sync.dma_start(MSHt[h][b * BLK:(b + 1) * BLK, :],
                                      msos[h * BLK:(h + 1) * BLK, :])
            assert ti == T

        if stage >= 2:
            edge_phase(KV1, Q1, MS1, MSH1)

        # ================= node phase =================
        def node_phase(MSt, MSHt, lhs_chunks, ws, bs, we, gt, bt, at_names, STI, STO, at_dt):
            nkc = len(lhs_chunks)
            for mt in range(NTS):
                sl = slice(mt * 128, (mt + 1) * 128)
                ms = msop.tile([128, 264], f32, name="msi")
                nc.sync.dma_start(ms[:, 0:256], MSt[sl, :])
                for h in range(2):
                    nc.sync.dma_start(ms[:, 256 + 2 * h:258 + 2 * h], MSHt[h][sl, :])
                ps = pj_tile()[:, 0:256]
                for kc in range(nkc):
                    nc.tensor.matmul(ps[:], lhsT=lhs_chunks[kc][:, sl],
                                     rhs=ws[:, kc * 256:(kc + 1) * 256],
                                     start=(kc == 0), stop=(kc == nkc - 1))
                den = smp.tile([128, 2], f32, name="den")
                for h in range(2):
                    nc.vector.tensor_scalar_add(out=den[:, h:h + 1],
                                                in0=ms[:, 256 + 2 * h:257 + 2 * h],
                                                scalar1=1e-16)
                rec = smp.tile([128, 2], f32, name="rec")
                nc.vector.reciprocal(out=rec[:], in_=den[:])
                tmp = scrp.tile([128, 256], f32, name="scr")
                for h in range(2):
                    hs = slice(h * 128, (h + 1) * 128)
                    nc.scalar.activation(out=tmp[:, hs], in_=we[:, hs],
                                         func=Act.Copy,
                                         scale=ms[:, 257 + 2 * h:258 + 2 * h])
                nc.vector.tensor_add(out=tmp[:], in0=tmp[:], in1=ms[:, 0:256])
                ho = hop.tile([128, 256], f32, name="ho")
                for h in range(2):
                    hs = slice(h * 128, (h + 1) * 128)
                    nc.scalar.activation(out=ho[:, hs], in_=tmp[:, hs],
                                         func=Act.Copy, scale=rec[:, h:h + 1])
                nc.vector.tensor_add(out=ho[:], in0=ho[:], in1=ps[:])
                if not zero_bias:
                    nc.vector.tensor_add(out=ho[:], in0=ho[:], in1=bs[:])
                for half in range(2):
                    tp = gpp.tile([128, 128], f32, name="tp", tag="gp")
                    nc.tensor.transpose(out=tp[:], in_=ho[:, half * 128:(half + 1) * 128],
                                        identity=ident[:])
                    nc.vector.tensor_copy(out=ht[half][:, sl], in_=tp[:])
            stt = smp.tile([128, 4], f32, name="stt")
            for half in range(2):
                nc.vector.reduce_sum(out=stt[:, half:half + 1], in_=ht[half][:, 0:NS],
                                     axis=AX.X)
                sq = bigp.tile([128, NSPAD], f32, name="sq")
                nc.scalar.activation(out=sq[:, 0:NS], in_=ht[half][:, 0:NS],
                                     func=Act.Square,
                                     accum_out=stt[:, 2 + half:3 + half])
            nc.sync.dma_start(STI[:, :], stt[:])
            nc.gpsimd.collective_compute(
                "AllGather", Alu.bypass, replica_groups=[list(range(R))],
                ins=[STI[:].opt()], outs=[STO[:].opt()])
            gsta = smp.tile([128, 4, R], f32, name="gsta")
            nc.sync.dma_start(
                gsta[:], STO[:].rearrange("(r p) k -> p k r", p=128))
            gst = smp.tile([128, 4], f32, name="gst")
            nc.vector.reduce_sum(out=gst[:], in_=gsta[:], axis=AX.X)
            mean = smp.tile([128, 2], f32, name="mean")
            nc.vector.tensor_scalar_mul(out=mean[:], in0=gst[:, 0:2], scalar1=1.0 / N)
            var = smp.tile([128, 2], f32, name="var")
            nc.vector.tensor_scalar_mul(out=var[:], in0=gst[:, 2:4], scalar1=1.0 / N)
            m2 = smp.tile([128, 2], f32, name="m2")
            nc.vector.tensor_mul(out=m2[:], in0=mean[:], in1=mean[:])
            nc.vector.tensor_sub(out=var[:], in0=var[:], in1=m2[:])
            nc.vector.tensor_scalar_add(out=var[:], in0=var[:], scalar1=EPS)
            sd = smp.tile([128, 2], f32, name="sd")
            nc.scalar.activation(out=sd[:], in_=var[:], func=Act.Sqrt)
            rsd = smp.tile([128, 2], f32, name="rsd")
            nc.vector.reciprocal(out=rsd[:], in_=sd[:])
            sc2 = smp.tile([128, 2], f32, name="sc2")
            nc.vector.tensor_mul(out=sc2[:], in0=gt[:], in1=rsd[:])
            sh2 = smp.tile([128, 2], f32, name="sh2")
            nc.vector.tensor_mul(out=sh2[:], in0=mean[:], in1=sc2[:])
            nc.vector.tensor_sub(out=sh2[:], in0=bt[:], in1=sh2[:])
            at = [atp.tile([128, NSPAD], at_dt, name=nm) for nm in at_names]
            for half in range(2):
                nc.scalar.activation(out=at[half][:], in_=ht[half][:], func=Act.Relu,
                                     scale=sc2[:, half:half + 1],
                                     bias=sh2[:, half:half + 1])
            return at

        if stage >= 3:
            at1 = node_phase(MS1, MSH1, [xts], w1["WS1"], b1["BS1"], we1,
                             g1t, b1tt, ["at10", "at11"], STI1, STO1, bf16)

        # ================= layer-2 projections (data-parallel + AllGather) ====
        if stage >= 4:
            for mt in range(NTS):
                sl = slice(mt * 128, (mt + 1) * 128)
                kvt = kvtp.tile([128, 512], bf16, name="kvt")
                for Wn, Bn, off in (("WK2", "BK2", 0), ("WV2", "BV2", 256)):
                    ps = pj_tile()[:, 0:256]
                    for kc in range(2):
                        nc.tensor.matmul(ps[:], lhsT=at1[kc][:, sl],
                                         rhs=w2[Wn][:, kc * 256:(kc + 1) * 256],
                                         start=(kc == 0), stop=(kc == 1))
                    evict(kvt[:, off:off + 256], ps[:], b2[Bn])
                if mt < NTS - 1:
                    nc.sync.dma_start(KV2S[sl, :], kvt[:])
                else:
                    nc.sync.dma_start(KV2S[mt * 128:NS, :], kvt[:NS - mt * 128, :])
            nc.gpsimd.collective_compute(
                "AllGather", mybir.AluOpType.bypass, replica_groups=[list(range(R))],
                ins=[KV2S[:].opt()], outs=[KV2F[:].opt()])

            q_proj(at1, w2["WQ2"], b2["BQ2"], we2, Q2)

        if stage >= 5:
            edge_phase(KV2F, Q2, MS2, MSH2)

        if stage >= 99:
            at2 = node_phase(MS2, MSH2, at1, w2["WS2"], b2["BS2"], we2,
                             g2t, b2tt, ["at20", "at21"], STI2, STO2, f32)
            for half in range(2):
                nc.sync.dma_start(OUTT[half * 128:(half + 1) * 128, :], at2[half][:])
        else:
            bail()

    nc.compile()
    return nc


def _pack_inputs(inputs, per_core):
    import ml_dtypes
    bfh = ml_dtypes.bfloat16
    x = np.asarray(inputs["x"], np.float32)
    xT = np.zeros((128, NFULL), np.float32)
    xT[:, :N] = x.T
    common = {"XTF": np.ascontiguousarray(xT).astype(bfh)}

    def bc(v):
        return np.ascontiguousarray(np.broadcast_to(
            np.asarray(v, np.float32).reshape(1, -1), (128, v.shape[-1])))

    def chunks2(w):
        w = np.asarray(w, np.float32)
        return np.ascontiguousarray(np.concatenate([w[0:128], w[128:256]], axis=1))

    for L, nm in ((1, "1"), (2, "2")):
        for key, wn in (("Wk", "WK"), ("Wv", "WV"), ("Wq", "WQ"), ("Ws", "WS")):
            w = np.asarray(inputs[key + nm], np.float32)
            if wn == "WQ":
                w = w * np.float32(INV)
            if L == 1:
                common[wn + nm] = np.ascontiguousarray(w).astype(bfh)
            else:
                common[wn + nm] = chunks2(w).astype(bfh)
        for key, bn in (("bk", "BK"), ("bv", "BV"), ("bq", "BQ"), ("bs", "BS")):
            b = np.asarray(inputs[key + nm], np.float32)
            if bn == "BQ":
                b = b * np.float32(INV)
            common[bn + nm] = bc(b)
        common["WE" + nm] = bc(np.asarray(inputs["We" + nm], np.float32).reshape(-1))
        common["G" + nm + "T"] = np.ascontiguousarray(
            np.asarray(inputs["g" + nm], np.float32).reshape(2, 128).T)
        common["B" + nm + "TT"] = np.ascontiguousarray(
            np.asarray(inputs["b" + nm], np.float32).reshape(2, 128).T)

    in_maps = []
    for r in range(R):
        m = dict(common)
        xs = np.zeros((128, NSPAD), np.float32)
        xs[:, :NS] = x[NS * r:NS * (r + 1)].T
        m["XTS"] = np.ascontiguousarray(xs).astype(bfh)
        m.update(per_core[r])
        in_maps.append(m)
    return in_maps


def kernel(**inputs):
    from concourse import bass_utils

    edge_index = np.asarray(inputs["edge_index"])
    edge_attr = np.asarray(inputs["edge_attr"], np.float32)
    zb = all(not np.any(np.asarray(inputs[k]))
             for k in ("bq1", "bk1", "bv1", "bs1", "bq2", "bk2", "bv2", "bs2"))
    key = (hash(edge_index.tobytes()), zb)
    if key not in _CACHE:
        per_core, n_b, T = _prepare(edge_index, edge_attr)
        nc = _build(T, n_b, zero_bias=zb)
        _CACHE[key] = (nc, per_core)
    nc, per_core = _CACHE[key]

    in_maps = _pack_inputs(inputs, per_core)
    import os
    trace = bool(int(os.environ.get("KBENCH_TRACE", "0")))
    res = bass_utils.run_bass_kernel_spmd(
        nc, in_maps, core_ids=list(range(R)), trace=trace)
    kernel.last_result = res
    out = np.concatenate(
        [res.results[r]["OUTT"][:, :NS].T for r in range(R)], axis=0)
    return np.ascontiguousarray(out)


# revision 45
# speedup vs baseline: 1.0296x; 1.0100x over previous
"""Trainium2 Bass kernel for the 2-layer TransformerConv GNN (edge-parallel, 8 cores).

Strategy (edge parallel, per sharding hint):
  - Sort edges by dst; shard nodes into 8 equal slices of 1250; each core owns
    all edges whose dst falls in its slice, so segment-softmax and scatter-add
    are core-local (no softmax-stat collectives needed).
  - Layer-1 K/V node projections are computed replicated on every core
    (input x is replicated); layer-2 K/V are data-parallel over nodes followed
    by an AllGather of the fused K|V table. BN statistics use a tiny AllReduce.
  - Per core, edges are grouped into 64-node blocks; each block's <=9 tiles of
    128 edges accumulate their segment-sums in PSUM via a 0/1 selection-matrix
    matmul, so no indirect scatter is needed.
  - Per 128-edge tile: K|V rows are gathered with one indirect DMA from the
    fused [N,512] table, q rows (+ precomputed t = (q . We)/sqrt(C)) with a
    second indirect DMA. Softmax skips the max-subtraction (logits are O(1))
    and normalization is deferred to the node phase:
       out_n = (sum_e ex_e * v_e + (sum_e ex_e*ea_e) * We) / (sum_e ex_e + 1e-16)
"""

import math

import numpy as np

N, E, H, C, D, F = 10000, 160000, 2, 128, 256, 128
R, NS, BLK = 8, 1250, 64
NB = (NS + BLK - 1) // BLK          # 20 blocks/core
NSPAD = NB * BLK                    # 1280
NFULL = 10112                       # 79*128, padded global node count
NT1 = NFULL // 128                  # 79
NTS = NSPAD // 128                  # 10
EPS = 1e-5
INV = 1.0 / math.sqrt(C)

_CACHE = {}


def _prepare(edge_index, edge_attr):
    """Host-side index preprocessing -> per-core tile arrays (data only)."""
    src = edge_index[0].astype(np.int32)
    dst = edge_index[1].astype(np.int32)
    ea = edge_attr[:, 0].astype(np.float32)
    perm = np.argsort(dst, kind="stable")
    sdst, ssrc, sea = dst[perm], src[perm], ea[perm]
    bounds = np.searchsorted(sdst, np.arange(0, N + 1, NS))
    core_info = []
    cnts = np.zeros((R, NB), dtype=np.int64)
    for r in range(R):
        lo, hi = bounds[r], bounds[r + 1]
        ldst = sdst[lo:hi] - NS * r
        bb = np.searchsorted(ldst, np.arange(0, NSPAD + 1, BLK))
        cnts[r] = np.diff(bb)
        core_info.append((lo, bb))
    n_b = np.maximum(1, np.ceil(cnts.max(axis=0) / 128).astype(np.int64))
    T = int(n_b.sum())
    per_core = []
    for r in range(R):
        lo, bb = core_info[r]
        SRC = np.zeros((T, 128), np.int32)
        DST = np.zeros((T, 128), np.int32)
        EAV = np.zeros((T, 128), np.float32)
        S = np.zeros((T, 128, BLK), np.float32)
        t = 0
        for b in range(NB):
            base = lo + bb[b]
            nb_edges = bb[b + 1] - bb[b]
            for i in range(n_b[b]):
                e0 = min(128 * i, nb_edges)
                e1 = min(128 * (i + 1), nb_edges)
                cnt = e1 - e0
                if cnt > 0:
                    sl = slice(base + e0, base + e1)
                    ld = sdst[sl] - NS * r
                    SRC[t, :cnt] = ssrc[sl]
                    DST[t, :cnt] = ld
                    EAV[t, :cnt] = sea[sl]
                    S[t, np.arange(cnt), ld - BLK * b] = 1.0
                DST[t, cnt:] = BLK * b
                t += 1
        import ml_dtypes
        bf16 = ml_dtypes.bfloat16
        Sne = np.zeros((T, 128, 128), np.float32)
        Sne[:, :BLK, :] = S.transpose(0, 2, 1)
        onesea = np.zeros((128, 2 * T), np.float32)
        onesea[:, 0::2] = 1.0
        onesea[:, 1::2] = EAV.T
        per_core.append(
            dict(
                SRC=np.ascontiguousarray(SRC.T),                       # [128, T]
                EAV=np.ascontiguousarray(EAV.T),                       # [128, T]
                SEN=np.ascontiguousarray(
                    S.transpose(1, 0, 2).reshape(128, T * BLK)).astype(bf16),
                SNE=np.ascontiguousarray(
                    Sne.transpose(1, 0, 2).reshape(128, T * 128)).astype(bf16),
                OSEA=np.ascontiguousarray(onesea).astype(bf16),
            )
        )
    return per_core, n_b, T


def _build(T, n_b, stage=99, zero_bias=False):
    """Build + schedule the (shared-across-cores) Bass program.

    stage: 1=L1 projections, 2=+edge1, 3=+node1 (AllReduce), 4=+proj2 (AllGather),
           5=+edge2, 99=full.
    """
    import concourse.bass as bass
    import concourse.mybir as mybir
    import concourse.tile as tile
    from concourse import bacc
    from concourse.masks import make_identity

    f32 = mybir.dt.float32
    bf16 = mybir.dt.bfloat16
    i32 = mybir.dt.int32
    Alu = mybir.AluOpType
    Act = mybir.ActivationFunctionType
    AX = mybir.AxisListType

    nc = bacc.Bacc("TRN2", target_bir_lowering=False, debug=False,
                   enable_asserts=False, num_devices=R)

    def din(name, shape, dtype=f32):
        return nc.dram_tensor(name, shape, dtype, kind="ExternalInput").ap()

    XTF = din("XTF", [128, NFULL], bf16)
    XTS = din("XTS", [128, NSPAD], bf16)
    SRC = din("SRC", [128, T], i32)
    EAV = din("EAV", [128, T])
    SEN = din("SEN", [128, T * BLK], bf16)
    SNE = din("SNE", [128, T * 128], bf16)
    OSEA = din("OSEA", [128, 2 * T], bf16)
    W1 = {k: din(k, [128, 256], bf16) for k in ("WK1", "WV1", "WQ1", "WS1")}
    B1 = {k: din(k, [128, 256]) for k in ("BK1", "BV1", "BQ1", "BS1")}
    WE1 = din("WE1", [128, 256])
    W2 = {k: din(k, [128, 512], bf16) for k in ("WK2", "WV2", "WQ2", "WS2")}
    B2 = {k: din(k, [128, 256]) for k in ("BK2", "BV2", "BQ2", "BS2")}
    WE2 = din("WE2", [128, 256])
    G1T, B1TT = din("G1T", [128, 2]), din("B1TT", [128, 2])
    G2T, B2TT = din("G2T", [128, 2]), din("B2TT", [128, 2])
    OUTT = nc.dram_tensor("OUTT", [256, NSPAD], f32, kind="ExternalOutput").ap()

    from contextlib import ExitStack

    with tile.TileContext(nc) as tc, ExitStack() as ctx:
        cp = ctx.enter_context(tc.tile_pool(name="const", bufs=1))
        dram = ctx.enter_context(tc.tile_pool(name="dram", bufs=1, space="DRAM"))
        lhsp = ctx.enter_context(tc.tile_pool(name="lhsp", bufs=4))
        kvtp = ctx.enter_context(tc.tile_pool(name="kvtp", bufs=4))
        qtp = ctx.enter_context(tc.tile_pool(name="qtp", bufs=4))
        qtbp = ctx.enter_context(tc.tile_pool(name="qtbp", bufs=3))
        kvgp = ctx.enter_context(tc.tile_pool(name="kvgp", bufs=12))
        shp = ctx.enter_context(tc.tile_pool(name="shp", bufs=12))
        scrp = ctx.enter_context(tc.tile_pool(name="scrp", bufs=12))
        smp = ctx.enter_context(tc.tile_pool(name="smp", bufs=12))
        msop = ctx.enter_context(tc.tile_pool(name="msop", bufs=8))
        hop = ctx.enter_context(tc.tile_pool(name="hop", bufs=4))
        htp = ctx.enter_context(tc.tile_pool(name="htp", bufs=1))
        atp = ctx.enter_context(tc.tile_pool(name="atp", bufs=1))
        bigp = ctx.enter_context(tc.tile_pool(name="bigp", bufs=1))
        gpp = ctx.enter_context(tc.tile_pool(name="gpp", bufs=2, space="PSUM"))
        segp = ctx.enter_context(tc.tile_pool(name="segp", bufs=2, space="PSUM"))
        sump = ctx.enter_context(tc.tile_pool(name="sump", bufs=2, space="PSUM"))

        def pj_tile():
            return gpp.tile([128, 264], f32, name="pj", tag="gp")

        def evict(out_ap, ps_ap, bias_tile):
            if zero_bias:
                nc.scalar.copy(out=out_ap, in_=ps_ap)
            else:
                nc.vector.tensor_add(out=out_ap, in0=ps_ap, in1=bias_tile[:])

        # ---- DRAM scratch ----
        KV1 = dram.tile([NFULL, 512], bf16, name="KV1")
        Q1 = dram.tile([NSPAD, 264], bf16, name="Q1")
        MS1 = dram.tile([NSPAD, 256], f32, name="MS1")
        MSH1 = [dram.tile([NSPAD, 2], f32, name=f"MSH1{h}") for h in range(2)]
        KV2S = dram.tile([NS, 512], bf16, name="KV2S")
        KV2F = dram.tile([N, 512], bf16, name="KV2F", addr_space="Shared")
        Q2 = dram.tile([NSPAD, 264], bf16, name="Q2")
        MS2 = dram.tile([NSPAD, 256], f32, name="MS2")
        MSH2 = [dram.tile([NSPAD, 2], f32, name=f"MSH2{h}") for h in range(2)]
        STI1 = dram.tile([128, 4], f32, name="STI1")
        STO1 = dram.tile([128 * R, 4], f32, name="STO1", addr_space="Shared")
        STI2 = dram.tile([128, 4], f32, name="STI2")
        STO2 = dram.tile([128 * R, 4], f32, name="STO2", addr_space="Shared")

        # ---- resident SBUF constants ----
        def load(name, ap, shape, dtype=f32):
            t = cp.tile(shape, dtype, name=name)
            nc.sync.dma_start(t[:], ap[:, :])
            return t

        xts = load("xts", XTS, [128, NSPAD], bf16)
        w1 = {k: load(k.lower(), v, [128, 256], bf16) for k, v in W1.items()}
        b1 = {k: load(k.lower(), v, [128, 256]) for k, v in B1.items()}
        we1 = load("we1", WE1, [128, 256])
        w2 = {k: load(k.lower(), v, [128, 512], bf16) for k, v in W2.items()}
        b2 = {k: load(k.lower(), v, [128, 256]) for k, v in B2.items()}
        we2 = load("we2", WE2, [128, 256])
        g1t = load("g1t", G1T, [128, 2])
        b1tt = load("b1tt", B1TT, [128, 2])
        g2t = load("g2t", G2T, [128, 2])
        b2tt = load("b2tt", B2TT, [128, 2])
        ident = cp.tile([128, 128], f32, name="ident")
        make_identity(nc, ident[:])

        ht = [htp.tile([128, NSPAD], f32, name=f"ht{h}") for h in range(2)]

        # ================= layer-1 projections =================
        # K/V for ALL nodes, replicated on every core.
        nt = 0
        lhs = None
        while nt < NT1:
            if nt % 8 == 0:
                nl = min(8, NT1 - nt)
                lhs = lhsp.tile([128, 8 * 128], bf16, name="lhs")
                nc.sync.dma_start(lhs[:, 0:nl * 128],
                                  XTF[:, nt * 128:(nt + nl) * 128])
                lbase = nt
            nj = min(4, NT1 - nt)
            kvt = kvtp.tile([128, 4 * 512], bf16, name="kvt")
            for j in range(nj):
                jj = nt - lbase + j
                for Wn, Bn, off in (("WK1", "BK1", 0), ("WV1", "BV1", 256)):
                    ps = pj_tile()[:, 0:256]
                    nc.tensor.matmul(ps[:], lhsT=lhs[:, jj * 128:(jj + 1) * 128],
                                     rhs=w1[Wn][:], start=True, stop=True)
                    evict(kvt[:, j * 512 + off:j * 512 + off + 256], ps[:], b1[Bn])
            nc.sync.dma_start(
                KV1[nt * 128:(nt + nj) * 128, :].rearrange("(j p) c -> p j c", p=128),
                kvt[:, 0:nj * 512].rearrange("p (j c) -> p j c", c=512))
            nt += nj

        def q_proj(lhs_chunks, wq, bq, we, Qd):
            for mt in range(NTS):
                sl = slice(mt * 128, (mt + 1) * 128)
                qt = qtp.tile([128, 264], bf16, name="qt")
                ps = pj_tile()[:, 0:256]
                nkc = len(lhs_chunks)
                for kc in range(nkc):
                    nc.tensor.matmul(ps[:], lhsT=lhs_chunks[kc][:, sl],
                                     rhs=wq[:, kc * 256:(kc + 1) * 256],
                                     start=(kc == 0), stop=(kc == nkc - 1))
                evict(qt[:, 0:256], ps[:], bq)
                scr = scrp.tile([128, 256], f32, name="scr")
                nc.vector.tensor_mul(out=scr[:], in0=qt[:, 0:256], in1=we[:, 0:256])
                tsum = smp.tile([128, 2], f32, name="tsum")
                nc.vector.reduce_sum(out=tsum[:],
                                     in_=scr[:].rearrange("p (h c) -> p h c", c=128),
                                     axis=AX.X)
                nc.vector.tensor_copy(out=qt[:, 256:258], in_=tsum[:])
                nc.sync.dma_start(Qd[sl, :], qt[:])

        q_proj([xts], w1["WQ1"], b1["BQ1"], we1, Q1)

        # edge-phase constants: loaded after projections are queued so the
        # projection operand DMAs aren't stuck behind these ~18MB transfers
        srcs = load("srcs", SRC, [128, T], i32)
        eavs = load("eavs", EAV, [128, T])
        sens = load("sens", SEN, [128, T * BLK], bf16)
        snes = load("snes", SNE, [128, T * 128], bf16)
        oseas = load("oseas", OSEA, [128, 2 * T], bf16)

        def bail():
            z = bigp.tile([128, NSPAD], f32, name="sq")
            nc.vector.memset(z[:], 0.0)
            for half in range(2):
                nc.sync.dma_start(OUTT[half * 128:(half + 1) * 128, :], z[:])

        # ================= edge phase =================
        def edge_phase(KVt, Qd, MSt, MSHt):
            kv_rows = KVt.shape[0]
            ti = 0
            for b in range(NB):
                msv = [segp.tile([BLK, 128], f32, name=f"msv{h}") for h in range(2)]
                mss = sump.tile([128, 2], f32, name="mss")
                qtb = qtbp.tile([128, 264], bf16, name="qtb")
                nc.vector.memset(qtb[BLK:128, :], 0.0)
                nc.sync.dma_start(qtb[0:BLK, :], Qd[b * BLK:(b + 1) * BLK, :])
                last = int(n_b[b]) - 1
                for i in range(int(n_b[b])):
                    t = ti
                    ti += 1
                    kvg = kvgp.tile([128, 512], bf16, name="kvg")
                    nc.gpsimd.indirect_dma_start(
                        out=kvg[:], out_offset=None, in_=KVt[:, :],
                        in_offset=bass.IndirectOffsetOnAxis(ap=srcs[:, t:t + 1], axis=0),
                        bounds_check=kv_rows - 1, oob_is_err=False)
                    qgps = gpp.tile([128, 264], f32, name="qgps", tag="gp")
                    nc.tensor.matmul(out=qgps[:, 0:258],
                                     lhsT=snes[:, t * 128:(t + 1) * 128],
                                     rhs=qtb[:, 0:258], start=True, stop=True)
                    scr = scrp.tile([128, 256], f32, name="scr")
                    asum = smp.tile([128, 2], f32, name="asum")
                    nc.vector.tensor_mul(out=scr[:], in0=qgps[:, 0:256],
                                         in1=kvg[:, 0:256])
                    nc.vector.reduce_sum(out=asum[:],
                                         in_=scr[:].rearrange("p (h c) -> p h c", c=128),
                                         axis=AX.X)
                    eat = smp.tile([128, 2], f32, name="eat")
                    nc.vector.tensor_mul(out=eat[:], in0=qgps[:, 256:258],
                                         in1=eavs[:, t:t + 1].to_broadcast([128, 2]))
                    nc.vector.tensor_add(out=eat[:], in0=eat[:], in1=asum[:])
                    ex = smp.tile([128, 2], f32, name="ex")
                    nc.scalar.activation(out=ex[:], in_=eat[:], func=Act.Exp)
                    sh = shp.tile([128, 128], bf16, name="sh")
                    for h in range(2):
                        nc.scalar.activation(out=sh[:, h * BLK:(h + 1) * BLK],
                                             in_=sens[:, t * BLK:(t + 1) * BLK],
                                             func=Act.Copy, scale=ex[:, h:h + 1])
                    for h in range(2):
                        nc.tensor.matmul(out=msv[h][:, 0:128],
                                         lhsT=sh[:, h * BLK:(h + 1) * BLK],
                                         rhs=kvg[:, 256 + h * 128:256 + (h + 1) * 128],
                                         start=(i == 0), stop=(i == last))
                    nc.tensor.matmul(out=mss[:, 0:2], lhsT=sh[:, 0:128],
                                     rhs=oseas[:, 2 * t:2 * t + 2],
                                     start=(i == 0), stop=(i == last))
                mso = msop.tile([BLK, 256], f32, name="mso")
                for h in range(2):
                    nc.any.tensor_copy(out=mso[:, h * 128:(h + 1) * 128],
                                       in_=msv[h][:, 0:128])
                msos = msop.tile([128, 2], f32, name="msos")
                nc.any.tensor_copy(out=msos[:], in_=mss[:])
                nc.sync.dma_start(MSt[b * BLK:(b + 1) * BLK, :], mso[:])
                for h in range(2):
                    nc.sync.dma_start(MSHt[h][b * BLK:(b + 1) * BLK, :],
                                      msos[h * BLK:(h + 1) * BLK, :])
            assert ti == T

        if stage >= 2:
            edge_phase(KV1, Q1, MS1, MSH1)

        # ================= node phase =================
        def node_phase(MSt, MSHt, lhs_chunks, ws, bs, we, gt, bt, at_names, STI, STO, at_dt):
            nkc = len(lhs_chunks)
            for mt in range(NTS):
                sl = slice(mt * 128, (mt + 1) * 128)
                ms = msop.tile([128, 264], f32, name="msi")
                nc.sync.dma_start(ms[:, 0:256], MSt[sl, :])
                for h in range(2):
                    nc.sync.dma_start(ms[:, 256 + 2 * h:258 + 2 * h], MSHt[h][sl, :])
                ps = pj_tile()[:, 0:256]
                for kc in range(nkc):
                    nc.tensor.matmul(ps[:], lhsT=lhs_chunks[kc][:, sl],
                                     rhs=ws[:, kc * 256:(kc + 1) * 256],
                                     start=(kc == 0), stop=(kc == nkc - 1))
                den = smp.tile([128, 2], f32, name="den")
                for h in range(2):
                    nc.vector.tensor_scalar_add(out=den[:, h:h + 1],
                                                in0=ms[:, 256 + 2 * h:257 + 2 * h],
                                                scalar1=1e-16)
                rec = smp.tile([128, 2], f32, name="rec")
                nc.vector.reciprocal(out=rec[:], in_=den[:])
                tmp = scrp.tile([128, 256], f32, name="scr")
                for h in range(2):
                    hs = slice(h * 128, (h + 1) * 128)
                    nc.scalar.activation(out=tmp[:, hs], in_=we[:, hs],
                                         func=Act.Copy,
                                         scale=ms[:, 257 + 2 * h:258 + 2 * h])
                nc.vector.tensor_add(out=tmp[:], in0=tmp[:], in1=ms[:, 0:256])
                ho = hop.tile([128, 256], f32, name="ho")
                for h in range(2):
                    hs = slice(h * 128, (h + 1) * 128)
                    nc.scalar.activation(out=ho[:, hs], in_=tmp[:, hs],
                                         func=Act.Copy, scale=rec[:, h:h + 1])
                nc.vector.tensor_add(out=ho[:], in0=ho[:], in1=ps[:])
                if not zero_bias:
                    nc.vector.tensor_add(out=ho[:], in0=ho[:], in1=bs[:])
                for half in range(2):
                    tp = gpp.tile([128, 128], f32, name="tp", tag="gp")
                    nc.tensor.transpose(out=tp[:], in_=ho[:, half * 128:(half + 1) * 128],
                                        identity=ident[:])
                    nc.vector.tensor_copy(out=ht[half][:, sl], in_=tp[:])
            stt = smp.tile([128, 4], f32, name="stt")
            for half in range(2):
                nc.vector.reduce_sum(out=stt[:, half:half + 1], in_=ht[half][:, 0:NS],
                                     axis=AX.X)
                sq = bigp.tile([128, NSPAD], f32, name="sq")
                nc.scalar.activation(out=sq[:, 0:NS], in_=ht[half][:, 0:NS],
                                     func=Act.Square,
                                     accum_out=stt[:, 2 + half:3 + half])
            nc.sync.dma_start(STI[:, :], stt[:])
            nc.gpsimd.collective_compute(
                "AllGather", Alu.bypass, replica_groups=[list(range(R))],
                ins=[STI[:].opt()], outs=[STO[:].opt()])
            gsta = smp.tile([128, 4, R], f32, name="gsta")
            nc.sync.dma_start(
                gsta[:], STO[:].rearrange("(r p) k -> p k r", p=128))
            gst = smp.tile([128, 4], f32, name="gst")
            nc.vector.reduce_sum(out=gst[:], in_=gsta[:], axis=AX.X)
            mean = smp.tile([128, 2], f32, name="mean")
            nc.vector.tensor_scalar_mul(out=mean[:], in0=gst[:, 0:2], scalar1=1.0 / N)
            var = smp.tile([128, 2], f32, name="var")
            nc.vector.tensor_scalar_mul(out=var[:], in0=gst[:, 2:4], scalar1=1.0 / N)
            m2 = smp.tile([128, 2], f32, name="m2")
            nc.vector.tensor_mul(out=m2[:], in0=mean[:], in1=mean[:])
            nc.vector.tensor_sub(out=var[:], in0=var[:], in1=m2[:])
            nc.vector.tensor_scalar_add(out=var[:], in0=var[:], scalar1=EPS)
            sd = smp.tile([128, 2], f32, name="sd")
            nc.scalar.activation(out=sd[:], in_=var[:], func=Act.Sqrt)
            rsd = smp.tile([128, 2], f32, name="rsd")
            nc.vector.reciprocal(out=rsd[:], in_=sd[:])
            sc2 = smp.tile([128, 2], f32, name="sc2")
            nc.vector.tensor_mul(out=sc2[:], in0=gt[:], in1=rsd[:])
            sh2 = smp.tile([128, 2], f32, name="sh2")
            nc.vector.tensor_mul(out=sh2[:], in0=mean[:], in1=sc2[:])
            nc.vector.tensor_sub(out=sh2[:], in0=bt[:], in1=sh2[:])
            at = [atp.tile([128, NSPAD], at_dt, name=nm) for nm in at_names]
            for half in range(2):
                nc.scalar.activation(out=at[half][:], in_=ht[half][:], func=Act.Relu,
                                     scale=sc2[:, half:half + 1],
                                     bias=sh2[:, half:half + 1])
            return at

        if stage >= 3:
            at1 = node_phase(MS1, MSH1, [xts], w1["WS1"], b1["BS1"], we1,
                             g1t, b1tt, ["at10", "at11"], STI1, STO1, bf16)

        # ================= layer-2 projections (data-parallel + AllGather) ====
        if stage >= 4:
            for mt in range(NTS):
                sl = slice(mt * 128, (mt + 1) * 128)
                kvt = kvtp.tile([128, 512], bf16, name="kvt")
                for Wn, Bn, off in (("WK2", "BK2", 0), ("WV2", "BV2", 256)):
                    ps = pj_tile()[:, 0:256]
                    for kc in range(2):
                        nc.tensor.matmul(ps[:], lhsT=at1[kc][:, sl],
                                         rhs=w2[Wn][:, kc * 256:(kc + 1) * 256],
                                         start=(kc == 0), stop=(kc == 1))
                    evict(kvt[:, off:off + 256], ps[:], b2[Bn])
                if mt < NTS - 1:
                    nc.sync.dma_start(KV2S[sl, :], kvt[:])
                else:
                    nc.sync.dma_start(KV2S[mt * 128:NS, :], kvt[:NS - mt * 128, :])
            nc.gpsimd.collective_compute(
                "AllGather", mybir.AluOpType.bypass, replica_groups=[list(range(R))],
                ins=[KV2S[:].opt()], outs=[KV2F[:].opt()])

            q_proj(at1, w2["WQ2"], b2["BQ2"], we2, Q2)

        if stage >= 5:
            edge_phase(KV2F, Q2, MS2, MSH2)

        if stage >= 99:
            at2 = node_phase(MS2, MSH2, at1, w2["WS2"], b2["BS2"], we2,
                             g2t, b2tt, ["at20", "at21"], STI2, STO2, f32)
            for half in range(2):
                nc.sync.dma_start(OUTT[half * 128:(half + 1) * 128, :], at2[half][:])
        else:
            bail()

    nc.compile()
    return nc


def _pack_inputs(inputs, per_core):
    import ml_dtypes
    bfh = ml_dtypes.bfloat16
    x = np.asarray(inputs["x"], np.float32)
    xT = np.zeros((128, NFULL), np.float32)
    xT[:, :N] = x.T
    common = {"XTF": np.ascontiguousarray(xT).astype(bfh)}

    def bc(v):
        return np.ascontiguousarray(np.broadcast_to(
            np.asarray(v, np.float32).reshape(1, -1), (128, v.shape[-1])))

    def chunks2(w):
        w = np.asarray(w, np.float32)
        return np.ascontiguousarray(np.concatenate([w[0:128], w[128:256]], axis=1))

    for L, nm in ((1, "1"), (2, "2")):
        for key, wn in (("Wk", "WK"), ("Wv", "WV"), ("Wq", "WQ"), ("Ws", "WS")):
            w = np.asarray(inputs[key + nm], np.float32)
            if wn == "WQ":
                w = w * np.float32(INV)
            if L == 1:
                common[wn + nm] = np.ascontiguousarray(w).astype(bfh)
            else:
                common[wn + nm] = chunks2(w).astype(bfh)
        for key, bn in (("bk", "BK"), ("bv", "BV"), ("bq", "BQ"), ("bs", "BS")):
            b = np.asarray(inputs[key + nm], np.float32)
            if bn == "BQ":
                b = b * np.float32(INV)
            common[bn + nm] = bc(b)
        common["WE" + nm] = bc(np.asarray(inputs["We" + nm], np.float32).reshape(-1))
        common["G" + nm + "T"] = np.ascontiguousarray(
            np.asarray(inputs["g" + nm], np.float32).reshape(2, 128).T)
        common["B" + nm + "TT"] = np.ascontiguousarray(
            np.asarray(inputs["b" + nm], np.float32).reshape(2, 128).T)

    in_maps = []
    for r in range(R):
        m = dict(common)
        xs = np.zeros((128, NSPAD), np.float32)
        xs[:, :NS] = x[NS * r:NS * (r + 1)].T
        m["XTS"] = np.ascontiguousarray(xs).astype(bfh)
        m.update(per_core[r])
        in_maps.append(m)
    return in_maps


def kernel(**inputs):
    from concourse import bass_utils

    edge_index = np.asarray(inputs["edge_index"])
    edge_attr = np.asarray(inputs["edge_attr"], np.float32)
    zb = all(not np.any(np.asarray(inputs[k]))
             for k in ("bq1", "bk1", "bv1", "bs1", "bq2", "bk2", "bv2", "bs2"))
    key = (hash(edge_index.tobytes()), zb)
    if key not in _CACHE:
        per_core, n_b, T = _prepare(edge_index, edge_attr)
        nc = _build(T, n_b, zero_bias=zb)
        _CACHE[key] = (nc, per_core)
    nc, per_core = _CACHE[key]

    in_maps = _pack_inputs(inputs, per_core)
    import os
    trace = bool(int(os.environ.get("KBENCH_TRACE", "0")))
    res = bass_utils.run_bass_kernel_spmd(
        nc, in_maps, core_ids=list(range(R)), trace=trace)
    kernel.last_result = res
    out = np.concatenate(
        [res.results[r]["OUTT"][:, :NS].T for r in range(R)], axis=0)
    return np.ascontiguousarray(out)
